# revision 3
# baseline (speedup 1.0000x reference)
"""Conditional (per-row expert) linear layer for Trainium2, 8 NeuronCores.

Math: out[i] = W[c_i] @ x[i] + sum_c b[c]    (x: [B,D], W: [C,D,D], b: [C,D])

Strategy: expert-parallel. Core c handles exactly the rows with
condition_ids == c (gathered on host, padded to a common capacity so the
SPMD NEFF has static shapes). Each core runs one [n_cap, D] @ [D, D] GEMM
with bf16 operands (fp32 PSUM accumulation), the host scatters rows back
and adds the (row-independent) summed bias in fp32. bf16 halves HBM
traffic vs fp32 (x 2.25MB + W 2MB + out 2.25MB per core) and enables the
PE's fast-weight-load path, so the kernel is bound by PE streaming:
n_cap*D*D/128^2 cycles @ 2.4GHz (~30.7us at n_cap=1152) plus per-k-tile
weight loads.
"""

import sys
from contextlib import ExitStack

import numpy as np
import ml_dtypes

try:
    import concourse.bass as bass  # noqa: F401
except ImportError:  # pragma: no cover
    sys.path.insert(0, "/opt/trn_rl_repo")

import jax
from jax.experimental.shard_map import shard_map
from jax.sharding import Mesh, PartitionSpec

import concourse.mybir as mybir
import concourse.tile as tile
from concourse import bacc
from concourse import bass2jax as _b2j

B, D, C = 8192, 1024, 8
P = 128  # partitions
KT = D // P  # k-tiles along the contraction dim
HALF = 512  # PSUM half-bank free size (fp32)
BF16 = ml_dtypes.bfloat16

_cache: dict[tuple, "_Runner"] = {}


def _build(n_cap: int, reps: int = 1):
    """Per-core program: out[n, o] = xT.T @ WT, n_cap x D output, bf16 io.

    reps > 1 repeats the whole body (including all DMAs) back-to-back for
    benchmarking: wall(T) - wall(1) isolates per-execution device time."""
    assert n_cap % 32 == 0
    row_tiles = [(s, min(P, n_cap - s)) for s in range(0, n_cap, P)]
    nc = bacc.Bacc("TRN2", target_bir_lowering=False, debug=False, num_devices=8, num_swdge_queues=4)
    xT = nc.dram_tensor("xT", [D, n_cap], mybir.dt.bfloat16, kind="ExternalInput").ap()
    WT = nc.dram_tensor("WT", [D, D], mybir.dt.bfloat16, kind="ExternalInput").ap()
    out = nc.dram_tensor("out", [n_cap, D], mybir.dt.bfloat16, kind="ExternalOutput").ap()

    with tile.TileContext(nc) as tc, ExitStack() as ctx:
        w_pool = ctx.enter_context(tc.tile_pool(name="w", bufs=2))
        x_pool = ctx.enter_context(tc.tile_pool(name="x", bufs=2))
        o_pool = ctx.enter_context(tc.tile_pool(name="o", bufs=1))
        ps_pool = ctx.enter_context(tc.tile_pool(name="ps", bufs=4, space="PSUM"))

        # Two k-groups with separate PSUM accumulations, combined on DVE.
        # A row-tile's group-A matmuls only need k0..k3 in SBUF, so the PE
        # starts long before the full input fill lands — the single-shot
        # fill overlaps the PE work instead of serializing it.
        k_groups = [range(0, KT // 2), range(KT // 2, KT)]
        xh = n_cap // 64 * 32  # x column split point (row dim), 32-aligned

        for _rep in range(reps):
            w_tiles, x_tiles = [], []
            for k in range(KT):
                wt = w_pool.tile([P, D], mybir.dt.bfloat16, name=f"wt{k}", tag=f"wt{k}")
                nc.sync.dma_start(wt[:, 0:HALF], WT[k * P : (k + 1) * P, 0:HALF])
                nc.sync.dma_start(wt[:, HALF:D], WT[k * P : (k + 1) * P, HALF:D])
                xt = x_pool.tile(
                    [P, n_cap], mybir.dt.bfloat16, name=f"xt{k}", tag=f"xt{k}"
                )
                nc.gpsimd.dma_start(xt[:, 0:xh], xT[k * P : (k + 1) * P, 0:xh])
                nc.gpsimd.dma_start(xt[:, xh:n_cap], xT[k * P : (k + 1) * P, xh:n_cap])
                w_tiles.append(wt)
                x_tiles.append(xt)

            o_tiles = {}
            for gi, ks in enumerate(k_groups):
                for start, size in row_tiles:
                    ps = ps_pool.tile([P, D], mybir.dt.float32, name="ps", tag="ps")
                    for k in ks:
                        for lo in (0, HALF):
                            nc.tensor.matmul(
                                ps[:size, lo : lo + HALF],
                                x_tiles[k][:, start : start + size],
                                w_tiles[k][:, lo : lo + HALF],
                                start=(k == ks[0]),
                                stop=(k == ks[-1]),
                                skip_group_check=True,
                            )
                    if gi == 0:
                        o_sb = o_pool.tile(
                            [P, D], mybir.dt.float32, name=f"o{start}", tag=f"o{start}"
                        )
                        o_tiles[start] = o_sb
                        nc.vector.tensor_copy(o_sb[:size, :], ps[:size, :])
                    else:
                        o_sb = o_tiles[start]
                        ob = o_pool.tile(
                            [P, D], mybir.dt.bfloat16, name=f"ob{start}", tag=f"ob{start}"
                        )
                        nc.vector.tensor_add(ob[:size, :], o_sb[:size, :], ps[:size, :])
                        out_eng = nc.sync if (start // P) % 2 == 0 else nc.gpsimd
                        out_eng.dma_start(out[start : start + size, :], ob[:size, :])

    nc.compile()
    _check_noload_pairs(nc)
    return nc


def _check_noload_pairs(nc):
    """Every ldweights=False matmul must execute (in PE stream order) with
    the stationary operand most recently loaded — by a standalone
    InstLdweights or by a self-loading matmul. Scheduling is deterministic
    at build time, so passing here guarantees correctness on device."""
    loaded = None
    for fn in nc.m.functions:
        for blk in fn.blocks:
            for inst in blk.instructions:
                tn = type(inst).__name__
                if tn == "InstLdweights":
                    loaded = str(inst.ins[0])
                elif tn == "InstMatmult":
                    if inst.ldweights is False:
                        assert loaded is not None, "no-load matmul with no load"
                        assert loaded == str(inst.ins[1]), (
                            f"no-load matmul stationary mismatch:\n"
                            f"loaded: {loaded}\nthis: {inst.ins[1]}"
                        )
                    else:
                        loaded = str(inst.ins[1])


class _Runner:
    """Caches the compiled NEFF + jitted shard_map executable for one n_cap."""

    def __init__(self, n_cap: int, reps: int = 1):
        self.n_cap = n_cap
        self.nc = _build(n_cap, reps)
        _b2j.install_neuronx_cc_hook()

        assert self.nc.dbg_addr is None
        partition_name = (
            self.nc.partition_id_tensor.name if self.nc.partition_id_tensor else None
        )

        in_names, out_names, out_avals = [], [], []
        for alloc in self.nc.m.functions[0].allocations:
            if not isinstance(alloc, mybir.MemoryLocationSet):
                continue
            name = alloc.memorylocations[0].name
            if alloc.kind == "ExternalInput":
                if name != partition_name:
                    in_names.append(name)
            elif alloc.kind == "ExternalOutput":
                out_names.append(name)
                out_avals.append(
                    jax.core.ShapedArray(
                        tuple(alloc.tensor_shape), mybir.dt.np(alloc.dtype)
                    )
                )
        self.in_names = in_names
        self.out_names = out_names
        self.out_avals = out_avals
        self.n_params = len(in_names)
        self.n_outs = len(out_names)
        all_in_names = tuple(in_names + out_names)
        if partition_name is not None:
            all_in_names = all_in_names + (partition_name,)

        nc = self.nc

        def _bind(*args):
            operands = list(args)
            if partition_name is not None:
                operands.append(_b2j.partition_id_tensor())
            return tuple(
                _b2j._bass_exec_p.bind(
                    *operands,
                    out_avals=tuple(out_avals),
                    in_names=all_in_names,
                    out_names=tuple(out_names),
                    lowering_input_output_aliases=(),
                    sim_require_finite=True,
                    sim_require_nnan=True,
                    nc=nc,
                )
            )

        self._bind = _bind
        self.devices = jax.devices("neuron")[:C]
        self.mesh = Mesh(np.asarray(self.devices), ("core",))
        spec_in = (PartitionSpec("core"),) * (self.n_params + self.n_outs)
        spec_out = (PartitionSpec("core"),) * self.n_outs
        self._spec_in, self._spec_out = spec_in, spec_out
        self._exec = jax.jit(
            shard_map(
                _bind,
                mesh=self.mesh,
                in_specs=spec_in,
                out_specs=spec_out,
                check_rep=False,
            ),
            donate_argnums=tuple(range(self.n_params, self.n_params + self.n_outs)),
            keep_unused=True,
        )

    def make_exec_nodonate(self):
        """Jitted executable that does not donate its output-init operands,
        so pre-staged device args can be reused across timing reps."""
        return jax.jit(
            shard_map(
                self._bind,
                mesh=self.mesh,
                in_specs=self._spec_in,
                out_specs=self._spec_out,
                check_rep=False,
            ),
            keep_unused=True,
        )

    def concat_inputs(self, in_maps):
        return [
            np.concatenate([np.asarray(m[name]) for m in in_maps], axis=0)
            for name in self.in_names
        ]

    def zero_outs(self):
        return [
            np.zeros((C * a.shape[0], *a.shape[1:]), a.dtype) for a in self.out_avals
        ]

    def run(self, in_maps):
        out_arrs = self._exec(*self.concat_inputs(in_maps), *self.zero_outs())
        return [
            {
                name: np.asarray(out_arrs[i]).reshape(C, *self.out_avals[i].shape)[c]
                for i, name in enumerate(self.out_names)
            }
            for c in range(C)
        ]


def _get(n_cap: int, reps: int = 1) -> _Runner:
    key = (n_cap, reps)
    if key not in _cache:
        _cache[key] = _Runner(n_cap, reps)
    return _cache[key]


def _prep(x, condition_ids, W, b):
    x = np.asarray(x, dtype=np.float32)
    cond = np.asarray(condition_ids).astype(np.int64)
    W = np.asarray(W, dtype=np.float32)
    b = np.asarray(b, dtype=np.float32)

    bias_sum = b.sum(axis=0, dtype=np.float32)

    rows = [np.nonzero(cond == c)[0] for c in range(C)]
    n_max = max(len(r) for r in rows)
    n_cap = max(32, -(-n_max // 32) * 32)

    in_maps = []
    for c in range(C):
        r = rows[c]
        xg = np.zeros((n_cap, D), BF16)
        xg[: len(r)] = x[r].astype(BF16)
        in_maps.append(
            {
                "xT": np.ascontiguousarray(xg.T),
                "WT": np.ascontiguousarray(W[c].T.astype(BF16)),
            }
        )
    return rows, n_cap, in_maps, bias_sum


def _run(x, condition_ids, W, b, trace=False):
    rows, n_cap, in_maps, bias_sum = _prep(x, condition_ids, W, b)
    runner = _get(n_cap)
    results = runner.run(in_maps)

    out = np.empty((B, D), np.float32)
    for c in range(C):
        r = rows[c]
        out[r] = results[c]["out"][: len(r)].astype(np.float32) + bias_sum
    return out, runner


def kernel(x, condition_ids, W, b):
    out, _ = _run(x, condition_ids, W, b)
    return out
